# revision 8
# baseline (speedup 1.0000x reference)
"""Trainium2 Bass kernel for nn_Net_15083925144150 (battle policy net).

Strategy
--------
Pure data parallelism: the B=512 battle batch is split across 8 NeuronCores
(64 battles each); all weights are replicated.

Host-side prep (per shard): big feature tensors are pre-transposed to
feature-major [K, rows] and K-tile-packed into [nblk, 128, ntile*blk] arrays so
each GEMM block loads with ONE large contiguous DMA (the kernel is DMA-issue
bound otherwise).  Weights are packed the same way ([128, nchunk*cols] with
k<128 chunks zero-padded).  Gathers (active_idx / switch_idx) and validity
masks become {0,1} / {bias,-inf} f32 mask tensors built on the host; on-device
they are cheap mask-multiplies (partition-broadcast via a k=1 matmul with a
ones vector).

Algebraic factoring: the move head computes
    relu(xm @ W_mo1) = relu(battle_full@W1a + mv@W1b + t*w1c)
so battle_full@W1a is computed once per battle (not 8x), and the switch head
computes relu(battle_full@Wsa + u0[m]@Wsb) densely for all 6 mons, dots with
W_so2, and gathers the resulting *scalars* by switch_idx (mask-multiply).

All compute is f32; matmul accumulation in PSUM f32.
"""

import numpy as np
from contextlib import ExitStack

try:
    import concourse.bass as bass
except ImportError:  # pragma: no cover
    import sys
    sys.path.insert(0, "/opt/trn_rl_repo")
    import concourse.bass as bass
import concourse.mybir as mybir
import concourse.tile as tile
from concourse import bacc
from concourse.bass_utils import run_bass_kernel_spmd

F32 = mybir.dt.float32
RELU = mybir.ActivationFunctionType.Relu
COPY = mybir.ActivationFunctionType.Copy

NCORES = 8
BC = 64            # battles per core
MU = BC * 12       # (b, side, mon) rows per core = 768
P = 128

# streams: name -> (K, rows-per-mon, block-cols)
STREAMS = {
    "ms":   (516, 4, 512),
    "pool": (516, 8, 512),
    "lk":   (516, 5, 480),
    "it":   (256, 2, 512),
    "ab":   (256, 2, 512),
    "br":   (256, 1, 384),
}

# u_in chunk layout: (row offset in W_user, k, chunk name)
U_CHUNKS = [
    (0, 89, "user"),
    (89, 128, "lk0"), (217, 128, "lk1"), (345, 128, "lk2"),
    (473, 128, "lk3"), (601, 128, "lk4"),
    (729, 128, "berry"),
    (857, 128, "msx"),
    (985, 128, "item"),
    (1113, 128, "ab"),
    (1241, 20, "types"),
    (1261, 20, "tera"),
]

# battle_full chunk layout: (row offset in W_mo1/W_so1, k, source key)
BF_CHUNKS = [(0, 9, ("battle", 0))]
for _s in range(2):
    _base = 9 + _s * 1041
    BF_CHUNKS.append((_base, 17, ("side", _s)))
    for _m in range(4):
        BF_CHUNKS.append((_base + 17 + _m * 128, 128, ("act", _s, _m)))
    for _m in range(4):
        BF_CHUNKS.append((_base + 529 + _m * 128, 128, ("umax", _s, _m)))
N_BF = len(BF_CHUNKS)          # 19
N_MO = N_BF + 1                # + mv chunk (W1b)
N_SO = N_BF + 4                # + 4 Wsb chunks


def _stream_geom(key):
    K, rpm, blk = STREAMS[key]
    ncols = MU * rpm
    nblk = ncols // blk
    assert nblk * blk == ncols
    nt = K // 128          # full 128-row k-tiles
    rem = K - nt * 128     # remainder rows
    return K, rpm, blk, ncols, nblk, nt, rem


def _build_nc():
    nc = bacc.Bacc()
    di = lambda name, shape: nc.dram_tensor(name, shape, F32, kind="ExternalInput")

    xP, xR = {}, {}
    for key in STREAMS:
        K, rpm, blk, ncols, nblk, nt, rem = _stream_geom(key)
        xP[key] = di(f"x_{key}", [nblk, 128, nt * blk])
        if rem:
            xR[key] = di(f"xr_{key}", [nblk, rem, blk])

    userT = di("userT", [89, MU])
    typesT = di("typesT", [20, MU])
    teraT = di("teraT", [20, MU])
    sideT = di("sideT", [17, BC * 2])
    battleT = di("battleT", [9, BC])
    mvT = di("mvT", [128, BC * 4])
    lkmask = di("lkmask", [1, MU * 5])
    berrymask = di("berrymask", [1, MU])
    actmask = di("actmask", [1, MU])
    selmask = di("selmask", [1, 6 * BC * 6])
    pen_mv = di("pen_mv", [1, BC * 8])
    pen_sw = di("pen_sw", [1, BC * 6])

    wmsP = di("wmsP", [128, 5 * 128])
    witP = di("witP", [128, 2 * 128])
    wabP = di("wabP", [128, 2 * 128])
    wuP = di("wuP", [128, len(U_CHUNKS) * 512])
    wmo1P = di("wmo1P", [128, N_MO * 512])
    wso1P = di("wso1P", [128, N_SO * 512])
    w123P = di("w123P", [128, 12])   # w1c[4] | wmo2[4] | wso2[4]
    biasP = di("biasP", [128, 15])   # bu[4] | bm1[4] | bs1[4] | bms | bit | bab

    out_d = nc.dram_tensor("out", [BC, 14], F32, kind="ExternalOutput")

    with tile.TileContext(nc) as tc, ExitStack() as ctx:
        kp = ctx.enter_context(tc.tile_pool(name="keep", bufs=1))

        ones1 = kp.tile([1, 128], F32, tag="ones1", name="ones1")
        nc.vector.memset(ones1[:], 1.0)
        bias_t = kp.tile([128, 15], F32, tag="biasP", name="biasP")
        nc.sync.dma_start(bias_t[:], biasP[:])
        BU, BM1, BS1, BMS, BIT, BAB = 0, 4, 8, 12, 13, 14

        actmask_sb = kp.tile([1, MU], F32, tag="actmask", name="actmask")
        nc.sync.dma_start(actmask_sb[:], actmask[:])
        sel_sb = kp.tile([1, 6 * BC * 6], F32, tag="sel", name="sel")
        nc.sync.dma_start(sel_sb[:], selmask[:])
        pen_mv_sb = kp.tile([1, BC * 8], F32, tag="penmv", name="penmv")
        nc.sync.dma_start(pen_mv_sb[:], pen_mv[:])
        pen_sw_sb = kp.tile([1, BC * 6], F32, tag="pensw", name="pensw")
        nc.sync.dma_start(pen_sw_sb[:], pen_sw[:])
        lkmask_sb = kp.tile([1, MU * 5], F32, tag="lkmask", name="lkmask")
        nc.sync.dma_start(lkmask_sb[:], lkmask[:])
        berrymask_sb = kp.tile([1, MU], F32, tag="berrymask", name="berrymask")
        nc.sync.dma_start(berrymask_sb[:], berrymask[:])

        uT = [kp.tile([128, MU], F32, tag=f"uT{m}", name=f"uT{m}") for m in range(4)]

        with tc.tile_pool(name="chunks", bufs=1) as cp, \
             tc.tile_pool(name="l1tmp", bufs=1) as mt:
            chunk = {
                name: cp.tile([k, MU], F32, tag=f"ch_{name}", name=f"ch_{name}")
                for (_, k, name) in U_CHUNKS
            }
            nc.sync.dma_start(chunk["user"][:], userT[:])
            nc.sync.dma_start(chunk["types"][:], typesT[:])
            nc.sync.dma_start(chunk["tera"][:], teraT[:])

            msmax_t = mt.tile([128, MU], F32, tag="msmax", name="msmax")
            pool_t = mt.tile([128, MU], F32, tag="poolacc", name="poolacc")
            item_r = mt.tile([128, MU], F32, tag="itemraw", name="itemraw")
            ab_r = mt.tile([128, MU], F32, tag="abraw", name="abraw")

            # ---------------- phase 1: level-1 GEMM streams ----------------
            with tc.tile_pool(name="w1", bufs=1) as w1p, \
                 tc.tile_pool(name="xs", bufs=3) as xp, \
                 tc.tile_pool(name="rt", bufs=2) as rp, \
                 tc.tile_pool(name="ps1", bufs=3, space="PSUM") as pp, \
                 tc.tile_pool(name="psm", bufs=2, space="PSUM") as ppm:

                wms_t = w1p.tile([128, 5 * 128], F32, tag="wms", name="wms")
                nc.sync.dma_start(wms_t[:], wmsP[:])
                wit_t = w1p.tile([128, 2 * 128], F32, tag="wit", name="wit")
                nc.sync.dma_start(wit_t[:], witP[:])
                wab_t = w1p.tile([128, 2 * 128], F32, tag="wab", name="wab")
                nc.sync.dma_start(wab_t[:], wabP[:])
                wstream = {"ms": wms_t, "pool": wms_t, "lk": wms_t,
                           "it": wit_t, "ab": wab_t, "br": wit_t}
                bstream = {"ms": BMS, "pool": BMS, "lk": BMS,
                           "it": BIT, "ab": BAB, "br": BIT}

                def stream(key, epilogue):
                    K, rpm, blk, ncols, nblk, nt, rem = _stream_geom(key)
                    wt = wstream[key]
                    bcol = bstream[key]
                    for b in range(nblk):
                        xb = xp.tile([128, nt * blk], F32, tag="xb", name="xb")
                        nc.sync.dma_start(xb[:], xP[key][b])
                        if rem:
                            xr = xp.tile([rem, blk], F32, tag="xr", name="xr")
                            nc.sync.dma_start(xr[:], xR[key][b])
                        ps = pp.tile([128, blk], F32, tag="ps", name="ps")
                        for i in range(nt):
                            nc.tensor.matmul(
                                ps[:], wt[0:128, i * 128:(i + 1) * 128],
                                xb[:, i * blk:(i + 1) * blk],
                                start=(i == 0), stop=(i == nt - 1 and not rem))
                        if rem:
                            nc.tensor.matmul(
                                ps[:], wt[0:rem, nt * 128:nt * 128 + 128],
                                xr[:], start=False, stop=True)
                        r = rp.tile([128, blk], F32, tag="r", name="r")
                        nc.scalar.activation(r[:], ps[:], RELU,
                                             bias=bias_t[:, bcol:bcol + 1])
                        epilogue(r, b * blk, blk)

                # moveset: max over 4 slots
                def ep_ms(r, c0, blk):
                    mon0 = c0 // 4
                    t1 = rp.tile([128, blk // 2], F32, tag="t1", name="t1")
                    nc.vector.tensor_max(t1[:], r[:, 0::2], r[:, 1::2])
                    nc.vector.tensor_max(msmax_t[:, mon0:mon0 + blk // 4],
                                         t1[:, 0::2], t1[:, 1::2])

                stream("ms", ep_ms)

                # movepool: mean over 8 (sum here, scale later)
                def ep_pool(r, c0, blk):
                    mon0 = c0 // 8
                    t1 = rp.tile([128, blk // 2], F32, tag="t1", name="t1")
                    nc.vector.tensor_add(t1[:], r[:, 0::2], r[:, 1::2])
                    t2 = rp.tile([128, blk // 4], F32, tag="t2", name="t2")
                    nc.vector.tensor_add(t2[:], t1[:, 0::2], t1[:, 1::2])
                    nc.vector.tensor_add(pool_t[:, mon0:mon0 + blk // 8],
                                         t2[:, 0::2], t2[:, 1::2])

                stream("pool", ep_pool)

                # lookup moves: relu * mask -> 5 chunk tiles
                def ep_lk(r, c0, blk):
                    mon0 = c0 // 5
                    nmon = blk // 5
                    mrep = ppm.tile([128, blk], F32, tag="mrep", name="mrep")
                    nc.tensor.matmul(mrep[:], ones1[:],
                                     lkmask_sb[0:1, c0:c0 + blk],
                                     start=True, stop=True)
                    for s in range(5):
                        nc.vector.tensor_mul(chunk[f"lk{s}"][:, mon0:mon0 + nmon],
                                             r[:, s::5], mrep[:, s::5])

                stream("lk", ep_lk)

                # items / abilities: mean over 2 (sum here, scale later)
                def ep_item(r, c0, blk):
                    mon0 = c0 // 2
                    nc.vector.tensor_add(item_r[:, mon0:mon0 + blk // 2],
                                         r[:, 0::2], r[:, 1::2])

                def ep_ab(r, c0, blk):
                    mon0 = c0 // 2
                    nc.vector.tensor_add(ab_r[:, mon0:mon0 + blk // 2],
                                         r[:, 0::2], r[:, 1::2])

                stream("it", ep_item)
                stream("ab", ep_ab)

                # last berry: relu * mask
                def ep_berry(r, c0, blk):
                    mrep = ppm.tile([128, blk], F32, tag="mrep", name="mrep")
                    nc.tensor.matmul(mrep[:], ones1[:],
                                     berrymask_sb[0:1, c0:c0 + blk],
                                     start=True, stop=True)
                    nc.vector.tensor_mul(chunk["berry"][:, c0:c0 + blk],
                                         r[:], mrep[:])

                stream("br", ep_berry)

                # finalize pooled chunks
                pool_s = mt.tile([128, MU], F32, tag="pools", name="pools")
                nc.scalar.activation(pool_s[:], pool_t[:], COPY, scale=0.125)
                nc.vector.tensor_max(chunk["msx"][:], msmax_t[:], pool_s[:])
                nc.scalar.activation(chunk["item"][:], item_r[:], COPY, scale=0.5)
                nc.scalar.activation(chunk["ab"][:], ab_r[:], COPY, scale=0.5)

            # ---------------- phase 2: user MLP (1281 -> 512) ----------------
            with tc.tile_pool(name="w2", bufs=1) as w2p, \
                 tc.tile_pool(name="ps2", bufs=4, space="PSUM") as pp2:
                wu_t = w2p.tile([128, len(U_CHUNKS) * 512], F32, tag="wu", name="wu")
                nc.sync.dma_start(wu_t[:], wuP[:])
                H = MU // 2
                for m in range(4):
                    for h in range(2):
                        pu = pp2.tile([128, H], F32, tag="pu", name="pu")
                        for i, (off, k, name) in enumerate(U_CHUNKS):
                            col = i * 512 + m * 128
                            nc.tensor.matmul(
                                pu[:], wu_t[0:k, col:col + 128],
                                chunk[name][:, h * H:(h + 1) * H],
                                start=(i == 0), stop=(i == len(U_CHUNKS) - 1))
                        nc.scalar.activation(uT[m][:, h * H:(h + 1) * H], pu[:],
                                             RELU, bias=bias_t[:, BU + m:BU + m + 1])

        # ---------------- phase 3: battle-level heads ----------------
        with tc.tile_pool(name="w3", bufs=1) as w3p, \
             tc.tile_pool(name="hd", bufs=1) as hp, \
             tc.tile_pool(name="ht", bufs=2) as htp, \
             tc.tile_pool(name="ps3", bufs=3, space="PSUM") as pp3, \
             tc.tile_pool(name="psa", bufs=2, space="PSUM") as ppa, \
             tc.tile_pool(name="psd", bufs=1, space="PSUM") as ppd:

            wmo1_t = w3p.tile([128, N_MO * 512], F32, tag="wmo1", name="wmo1")
            nc.sync.dma_start(wmo1_t[:], wmo1P[:])
            wso1_t = w3p.tile([128, N_SO * 512], F32, tag="wso1", name="wso1")
            nc.sync.dma_start(wso1_t[:], wso1P[:])
            w123_t = hp.tile([128, 12], F32, tag="w123", name="w123")
            nc.sync.dma_start(w123_t[:], w123P[:])

            # u.max over mons and active-gather (mask-multiply), per 128-feat tile
            umax_t, act_t, u0_t = [], [], []
            for m in range(4):
                x1 = htp.tile([128, MU // 2], F32, tag="x1", name="x1")
                nc.vector.tensor_max(x1[:], uT[m][:, 0::2], uT[m][:, 1::2])
                x2 = htp.tile([128, MU // 6], F32, tag="x2", name="x2")
                nc.vector.tensor_max(x2[:], x1[:, 0::3], x1[:, 1::3])
                um = hp.tile([128, MU // 6], F32, tag=f"umax{m}", name=f"umax{m}")
                nc.vector.tensor_max(um[:], x2[:], x1[:, 2::3])
                umax_t.append(um)

                arep = ppa.tile([128, MU // 2], F32, tag="arep", name="arep")
                am1 = htp.tile([128, MU], F32, tag="am1", name="am1")
                for h in range(2):
                    nc.tensor.matmul(arep[:], ones1[:],
                                     actmask_sb[0:1, h * 384:(h + 1) * 384],
                                     start=True, stop=True)
                    nc.vector.tensor_mul(am1[:, h * 384:(h + 1) * 384],
                                         uT[m][:, h * 384:(h + 1) * 384], arep[:])
                s1 = htp.tile([128, MU // 2], F32, tag="s1", name="s1")
                nc.vector.tensor_add(s1[:], am1[:, 0::2], am1[:, 1::2])
                s2 = htp.tile([128, MU // 6], F32, tag="s2", name="s2")
                nc.vector.tensor_add(s2[:], s1[:, 0::3], s1[:, 1::3])
                ac = hp.tile([128, MU // 6], F32, tag=f"act{m}", name=f"act{m}")
                nc.vector.tensor_add(ac[:], s2[:], s1[:, 2::3])
                act_t.append(ac)

                # side-0 u columns, materialized [128, BC*6]
                u0 = hp.tile([128, BC * 6], F32, tag=f"u0{m}", name=f"u0{m}")
                nc.vector.tensor_copy(
                    u0[:].rearrange("p (b m2) -> p b m2", m2=6),
                    uT[m][:].rearrange("p (b s m2) -> p b s m2", s=2, m2=6)[:, :, 0, :])
                u0_t.append(u0)

            sideT_sb = hp.tile([17, BC * 2], F32, tag="sideT", name="sideT")
            nc.sync.dma_start(sideT_sb[:], sideT[:])
            battleT_sb = hp.tile([9, BC], F32, tag="battleT", name="battleT")
            nc.sync.dma_start(battleT_sb[:], battleT[:])
            mvT_sb = hp.tile([128, BC * 4], F32, tag="mvT", name="mvT")
            nc.sync.dma_start(mvT_sb[:], mvT[:])

            def bf_rhs(srckey):
                kind = srckey[0]
                if kind == "battle":
                    return battleT_sb[:]
                if kind == "side":
                    return sideT_sb[:, srckey[1]::2]
                if kind == "act":
                    return act_t[srckey[2]][:, srckey[1]::2]
                return umax_t[srckey[2]][:, srckey[1]::2]

            # dense per-battle head matmuls
            h_t, hs_t, g_t, v_t = [], [], [], []
            for m in range(4):
                ph = pp3.tile([128, BC], F32, tag="hps", name="ph")
                for i, (off, k, srckey) in enumerate(BF_CHUNKS):
                    col = i * 512 + m * 128
                    nc.tensor.matmul(ph[:], wmo1_t[0:k, col:col + 128], bf_rhs(srckey),
                                     start=(i == 0), stop=(i == N_BF - 1))
                ht = hp.tile([128, BC], F32, tag=f"h{m}", name=f"h{m}")
                nc.vector.tensor_copy(ht[:], ph[:])
                h_t.append(ht)

                phs = pp3.tile([128, BC], F32, tag="hps", name="phs")
                for i, (off, k, srckey) in enumerate(BF_CHUNKS):
                    col = i * 512 + m * 128
                    nc.tensor.matmul(phs[:], wso1_t[0:k, col:col + 128], bf_rhs(srckey),
                                     start=(i == 0), stop=(i == N_BF - 1))
                hst = hp.tile([128, BC], F32, tag=f"hs{m}", name=f"hs{m}")
                nc.vector.tensor_copy(hst[:], phs[:])
                hs_t.append(hst)

                pg = pp3.tile([128, BC * 4], F32, tag="hps", name="pg")
                col = N_BF * 512 + m * 128
                nc.tensor.matmul(pg[:], wmo1_t[0:128, col:col + 128], mvT_sb[:],
                                 start=True, stop=True)
                gt = hp.tile([128, BC * 4], F32, tag=f"g{m}", name=f"g{m}")
                nc.vector.tensor_copy(gt[:], pg[:])
                g_t.append(gt)

                pv = pp3.tile([128, BC * 6], F32, tag="hps", name="pv")
                for s2 in range(4):
                    col = (N_BF + s2) * 512 + m * 128
                    nc.tensor.matmul(pv[:], wso1_t[0:128, col:col + 128], u0_t[s2][:],
                                     start=(s2 == 0), stop=(s2 == 3))
                vt = hp.tile([128, BC * 6], F32, tag=f"v{m}", name=f"v{m}")
                nc.vector.tensor_copy(vt[:], pv[:])
                v_t.append(vt)

            # move head: a[:, b,i,t] = h + g_i + t*w1c -> relu -> dot W_mo2
            rm_t, rs_t = [], []
            for m in range(4):
                h1 = htp.tile([128, BC], F32, tag="h1", name="h1")
                nc.vector.tensor_scalar_add(h1[:], h_t[m][:], w123_t[:, m:m + 1])
                am = htp.tile([128, BC * 8], F32, tag="am", name="am")
                for i in range(4):
                    nc.vector.tensor_add(am[:, (i * 2)::8], h_t[m][:], g_t[m][:, i::4])
                    nc.vector.tensor_add(am[:, (i * 2 + 1)::8], h1[:], g_t[m][:, i::4])
                rm = hp.tile([128, BC * 8], F32, tag=f"rm{m}", name=f"rm{m}")
                nc.scalar.activation(rm[:], am[:], RELU,
                                     bias=bias_t[:, BM1 + m:BM1 + m + 1])
                rm_t.append(rm)

                asw = htp.tile([128, BC * 6], F32, tag="asw", name="asw")
                for mm in range(6):
                    nc.vector.tensor_add(asw[:, mm::6], hs_t[m][:], v_t[m][:, mm::6])
                rs = hp.tile([128, BC * 6], F32, tag=f"rs{m}", name=f"rs{m}")
                nc.scalar.activation(rs[:], asw[:], RELU,
                                     bias=bias_t[:, BS1 + m:BS1 + m + 1])
                rs_t.append(rs)

            pml = ppd.tile([1, BC * 8], F32, tag="pml", name="pml")
            for m in range(4):
                nc.tensor.matmul(pml[:], w123_t[0:128, 4 + m:5 + m], rm_t[m][:],
                                 start=(m == 0), stop=(m == 3))
            mlog = hp.tile([1, BC * 8], F32, tag="mlog", name="mlog")
            nc.vector.tensor_add(mlog[:], pml[:], pen_mv_sb[:])

            pc = ppd.tile([1, BC * 6], F32, tag="pc", name="pc")
            for m in range(4):
                nc.tensor.matmul(pc[:], w123_t[0:128, 8 + m:9 + m], rs_t[m][:],
                                 start=(m == 0), stop=(m == 3))
            c_sb = hp.tile([1, BC * 6], F32, tag="c", name="c")
            nc.vector.tensor_copy(c_sb[:], pc[:])

            # slogit[b,j] = c[b, switch_idx[b,j]] via mask-multiply over mons
            slog = hp.tile([1, BC * 6], F32, tag="slog", name="slog")
            for j in range(6):
                tmp = htp.tile([1, BC * 6], F32, tag="seltmp", name="seltmp")
                nc.vector.tensor_mul(tmp[:], c_sb[:],
                                     sel_sb[0:1, j * BC * 6:(j + 1) * BC * 6])
                r1 = htp.tile([1, BC * 3], F32, tag="selr1", name="selr1")
                nc.vector.tensor_add(r1[:], tmp[:, 0::2], tmp[:, 1::2])
                r2 = htp.tile([1, BC], F32, tag="selr2", name="selr2")
                nc.vector.tensor_add(r2[:], r1[:, 0::3], r1[:, 1::3])
                nc.vector.tensor_add(slog[:, j::6], r2[:], r1[:, 2::3])
            slog2 = hp.tile([1, BC * 6], F32, tag="slog2", name="slog2")
            nc.vector.tensor_add(slog2[:], slog[:], pen_sw_sb[:])

            nc.sync.dma_start(
                out_d[:, 0:8].unsqueeze(0),
                mlog[0:1, :].rearrange("p (b k) -> p b k", k=8))
            nc.sync.dma_start(
                out_d[:, 8:14].unsqueeze(0),
                slog2[0:1, :].rearrange("p (b k) -> p b k", k=6))

    nc.compile()
    return nc


_NC_CACHE = []


def _get_nc():
    if not _NC_CACHE:
        _NC_CACHE.append(_build_nc())
    return _NC_CACHE[0]


def _pack_stream(xT, key):
    """xT [K, ncols] -> packed [nblk, 128, nt*blk] (+ remainder [nblk, rem, blk])."""
    K, rpm, blk, ncols, nblk, nt, rem = _stream_geom(key)
    f = np.float32
    full = xT[: nt * 128].reshape(nt, 128, nblk, blk)
    packed = np.ascontiguousarray(
        full.transpose(2, 1, 0, 3).reshape(nblk, 128, nt * blk)).astype(f, copy=False)
    out = {f"x_{key}": packed}
    if rem:
        r = xT[nt * 128:].reshape(rem, nblk, blk)
        out[f"xr_{key}"] = np.ascontiguousarray(r.transpose(1, 0, 2)).astype(f, copy=False)
    return out


def _pack_cols(chunks):
    """chunks: list of [k, w] arrays -> [128, sum(w)] with zero row-padding."""
    f = np.float32
    w_total = sum(c.shape[1] for c in chunks)
    out = np.zeros((128, w_total), f)
    col = 0
    for c in chunks:
        out[: c.shape[0], col:col + c.shape[1]] = c
        col += c.shape[1]
    return out


def make_in_maps(inputs):
    f = np.float32

    def T(a, K):
        return np.ascontiguousarray(np.asarray(a).reshape(-1, K).T).astype(f, copy=False)

    W_ms = np.asarray(inputs["W_ms"], f)
    W_item = np.asarray(inputs["W_item"], f)
    W_ab = np.asarray(inputs["W_ab"], f)
    W_user = np.asarray(inputs["W_user"], f)
    W_mo1 = np.asarray(inputs["W_mo1"], f)
    W_so1 = np.asarray(inputs["W_so1"], f)
    W_mo2 = np.asarray(inputs["W_mo2"], f)
    W_so2 = np.asarray(inputs["W_so2"], f)
    b_mo2 = float(np.asarray(inputs["b_mo2"]).reshape(-1)[0])
    b_so2 = float(np.asarray(inputs["b_so2"]).reshape(-1)[0])

    shared = {}
    shared["wmsP"] = _pack_cols([W_ms[i * 128:min((i + 1) * 128, 516)] for i in range(5)])
    shared["witP"] = _pack_cols([W_item[0:128], W_item[128:256]])
    shared["wabP"] = _pack_cols([W_ab[0:128], W_ab[128:256]])
    shared["wuP"] = _pack_cols([W_user[off:off + k] for (off, k, _) in U_CHUNKS])
    mo = [W_mo1[off:off + k] for (off, k, _) in BF_CHUNKS] + [W_mo1[2091:2219]]
    shared["wmo1P"] = _pack_cols(mo)
    so = [W_so1[off:off + k] for (off, k, _) in BF_CHUNKS] + \
         [W_so1[2091 + s * 128:2091 + (s + 1) * 128] for s in range(4)]
    shared["wso1P"] = _pack_cols(so)
    w123 = np.zeros((128, 12), f)
    for m in range(4):
        w123[:, m] = W_mo1[2219, m * 128:(m + 1) * 128]
        w123[:, 4 + m] = W_mo2[m * 128:(m + 1) * 128, 0]
        w123[:, 8 + m] = W_so2[m * 128:(m + 1) * 128, 0]
    shared["w123P"] = w123
    biasP = np.zeros((128, 15), f)
    for m in range(4):
        biasP[:, m] = np.asarray(inputs["b_user"], f)[m * 128:(m + 1) * 128]
        biasP[:, 4 + m] = np.asarray(inputs["b_mo1"], f)[m * 128:(m + 1) * 128]
        biasP[:, 8 + m] = np.asarray(inputs["b_so1"], f)[m * 128:(m + 1) * 128]
    biasP[:, 12] = np.asarray(inputs["b_ms"], f)
    biasP[:, 13] = np.asarray(inputs["b_item"], f)
    biasP[:, 14] = np.asarray(inputs["b_ab"], f)
    shared["biasP"] = biasP

    in_maps = []
    for c in range(NCORES):
        sl = slice(c * BC, (c + 1) * BC)
        m = dict(shared)
        m.update(_pack_stream(T(inputs["moveset_feat"][sl], 516), "ms"))
        m.update(_pack_stream(T(inputs["movepool_feat"][sl], 516), "pool"))
        m.update(_pack_stream(T(inputs["lookup_move_feat"][sl], 516), "lk"))
        m.update(_pack_stream(T(inputs["items"][sl], 256), "it"))
        m.update(_pack_stream(T(inputs["abilities"][sl], 256), "ab"))
        m.update(_pack_stream(T(inputs["lastberry"][sl], 256), "br"))
        m["userT"] = T(inputs["user_x"][sl], 89)
        m["typesT"] = T(inputs["types_x"][sl], 20)
        m["teraT"] = T(inputs["tera_types_x"][sl], 20)
        m["sideT"] = T(inputs["side_x"][sl], 17)
        m["battleT"] = T(inputs["battle_x"][sl], 9)
        m["mvT"] = T(inputs["opt_moves"][sl], 128)
        m["lkmask"] = np.ascontiguousarray(
            np.asarray(inputs["lookup_move_mask"][sl], f)).reshape(1, MU * 5)
        m["berrymask"] = np.ascontiguousarray(
            np.asarray(inputs["lastberry_mask"][sl], f)).reshape(1, MU)

        ai = np.asarray(inputs["active_idx"][sl]).astype(np.int64)  # [BC,2]
        A = np.zeros((BC, 2, 6), f)
        A[np.arange(BC)[:, None], np.arange(2)[None, :], ai] = 1.0
        m["actmask"] = A.reshape(1, MU)

        si = np.asarray(inputs["switch_idx"][sl]).astype(np.int64)  # [BC,6]
        S = np.zeros((6, BC, 6), f)
        for j in range(6):
            S[j, np.arange(BC), si[:, j]] = 1.0
        m["selmask"] = S.reshape(1, 6 * BC * 6)

        mmv = np.asarray(inputs["move_mask"][sl]) > 0  # [BC,4]
        ct = np.asarray(inputs["can_tera"][sl]) > 0  # [BC]
        tera_ok = np.stack([np.ones_like(ct), ct], axis=-1)  # [BC,2]
        valid = mmv[:, :, None] & tera_ok[:, None, :]  # [BC,4,2]
        m["pen_mv"] = np.where(valid, b_mo2, -np.inf).astype(f).reshape(1, BC * 8)
        swm = np.asarray(inputs["switch_mask"][sl]) > 0
        m["pen_sw"] = np.where(swm, b_so2, -np.inf).astype(f).reshape(1, BC * 6)
        in_maps.append(m)
    return in_maps


def kernel(**inputs):
    nc = _get_nc()
    in_maps = make_in_maps(inputs)
    res = run_bass_kernel_spmd(nc, in_maps, core_ids=list(range(NCORES)))
    outs = [np.asarray(r["out"]) for r in res.results]
    return np.concatenate(outs, axis=0).astype(np.float32)
